# revision 2
# baseline (speedup 1.0000x reference)
"""AttentionMIL pooling kernel v4p: host-side transposed x, no PE transposes.

v3 spent most of its device time on the T (PE transpose) + C (DVE PSUM->SBUF
copy) stages and their cross-engine round trips.  v4 removes both: the host
supplies x twice -- row-major (xn, with ones column, for the pooling matmul)
and feature-major (xt, for the attention MLP).  DMA measured essentially free
at these sizes (hidden under the per-exec dispatch overhead), while T+C cost
~0.4ms of device time.

Stage dataflow per chunk (4 subtiles of 128 rows = 512 rows):
  H:  h_ps[a, n] = sum_d W1[d,a] xt[d, n]   (2 K=128 halves into one bank)
  Th: ACT tanh(h_ps + b1) -> th [A, 512] fp16
  S': 4x PE matmuls lhsT=th[:, j*128:(j+1)*128], rhs=w2 [A,1]
      -> st_ps[q, j] = score of (subtile j, row q)  [128, 4]
      (replaces v3's S+G: scores land per-partition directly, no [1,512]
       lane-starved ops, no PSUM e-row transpose)
  E:  ACT exp(st_ps) -> et [128, 4] fp32
  L:  DVE tensor_scalar: sel[q, j, b] = (iota[q,b]==seg[q,t]) * et[q,j]
  U:  4x PE matmuls: u_ps += sel[:,j,:].T @ xn_subtile [128, 257]
      (ones column -> col 256 accumulates the softmax denominator)

Software pipelining: iteration c emits L(c-4) | H(c) Th(c-1) S'(c-2)
E(c-3) U(c-5) so every cross-engine edge has >= 1 chunk of slack.

Host: sum (U, den) partials over cores, pooled = U/den, out = pooled@Wh+bh.
"""

import numpy as np

import concourse.mybir as mybir
import concourse.tile as tile
from concourse import bacc
from concourse.bass_utils import run_bass_kernel_spmd

F16 = np.float16

N_CORES = 8
N_TOTAL = 524288
D = 256
DP = D + 1  # x padded with ones column
A = 128
B = 64  # num bags
P = 128  # SBUF partitions
R = N_TOTAL // N_CORES  # rows per core
T = R // P  # 512 subtiles of 128 rows per core
S = 32  # subtiles per super tile (xn DMA 2.1 MiB, xt DMA 2.0 MiB)
SUPERS = T // S
CH = 4  # subtiles per chunk (512 rows = PSUM bank width)
TOTCH = T // CH  # 128 chunks per core

_NC_CACHE = {}


def build_nc(R=R, S=S, xn_bufs=3, xt_bufs=3, n_cores=N_CORES, debug=False):
    T = R // P
    SUPERS = T // S
    TOTCH = T // CH
    CPS = S // CH  # chunks per super
    W = S * P  # xt columns per super
    dt = mybir.dt
    nc = bacc.Bacc("TRN2", target_bir_lowering=False, debug=debug, num_devices=n_cores)

    xn_d = nc.dram_tensor("xn", [R, DP], dt.float16, kind="ExternalInput")
    xt_d = nc.dram_tensor("xt", [2 * P, R], dt.float16, kind="ExternalInput")
    seg_d = nc.dram_tensor("seg", [P, T], dt.float32, kind="ExternalInput")
    w1_d = nc.dram_tensor("w1", [D, A], dt.float16, kind="ExternalInput")
    w2_d = nc.dram_tensor("w2", [A, 1], dt.float16, kind="ExternalInput")
    b1_d = nc.dram_tensor("b1", [A, 1], dt.float16, kind="ExternalInput")
    iota_d = nc.dram_tensor("iota", [P, B], dt.float16, kind="ExternalInput")
    uout_d = nc.dram_tensor("uout", [B, DP], dt.float32, kind="ExternalOutput")

    xn_view = xn_d.ap().rearrange("(s p a) d -> s p (a d)", p=P, a=S)
    xt_view4 = xt_d.ap().rearrange("(h p) (s w) -> s p h w", p=P, s=SUPERS)
    S4 = S // 4  # first super split into 4 quarter DMAs: compute starts sooner
    W4 = W // 4
    w1_view = w1_d.ap().rearrange("(h p) a -> p h a", p=P)

    with tile.TileContext(nc) as tc:
        with (
            tc.tile_pool(name="persist", bufs=1) as persist,
            tc.tile_pool(name="xn_pool", bufs=xn_bufs) as xn_pool,
            tc.tile_pool(name="xt_pool", bufs=xt_bufs) as xt_pool,
            tc.tile_pool(name="th_pool", bufs=3) as th_pool,
            tc.tile_pool(name="et_pool", bufs=3) as et_pool,
            tc.tile_pool(name="sel_pool", bufs=3) as sel_pool,
            tc.tile_pool(name="out_pool", bufs=1) as out_pool,
            tc.tile_pool(name="psum_u", bufs=1, space="PSUM") as psum_u,
            tc.tile_pool(name="psum_h", bufs=2, space="PSUM") as psum_h,
            tc.tile_pool(name="psum_st", bufs=2, space="PSUM") as psum_st,
        ):
            w1_sb = persist.tile([P, 2, A], dt.float16)
            nc.sync.dma_start(out=w1_sb, in_=w1_view)
            w2_sb = persist.tile([A, 1], dt.float16)
            nc.sync.dma_start(out=w2_sb, in_=w2_d.ap())
            b1_sb = persist.tile([A, 1], dt.float16)
            nc.sync.dma_start(out=b1_sb, in_=b1_d.ap())
            iota_sb = persist.tile([P, B], dt.float16)
            nc.sync.dma_start(out=iota_sb, in_=iota_d.ap())
            seg_sb = persist.tile([P, T], dt.float32)
            nc.sync.dma_start(out=seg_sb, in_=seg_d.ap())

            u_ps = psum_u.tile([B, DP], dt.float32)

            xns = {}  # super idx -> xn tile
            xts = {}  # super idx -> xt tile
            xn0 = []  # super 0 quarter tiles
            xt0 = []
            for k in range(4):
                q = persist.tile([P, S4 * DP], dt.float16, name=f"xn0_{k}")
                nc.sync.dma_start(
                    out=q, in_=xn_view[0][:, k * S4 * DP : (k + 1) * S4 * DP]
                )
                xn0.append(q)
                qt = persist.tile([P, 2, W4], dt.float16, name=f"xt0_{k}")
                nc.sync.dma_start(
                    out=qt, in_=xt_view4[0][:, :, k * W4 : (k + 1) * W4]
                )
                xt0.append(qt)
            st = {}  # chunk idx -> per-stage state

            def chunk_src(c):
                """(xn tile, subtile off, xt tile, col off) for chunk c."""
                sidx = c // CPS
                cc = c % CPS
                if sidx == 0:
                    per_q = CPS // 4  # chunks per quarter of super 0
                    qq = cc // per_q
                    a0 = (cc % per_q) * CH
                    return xn0[qq], a0, xt0[qq], a0 * P
                if sidx not in xns:
                    xn = xn_pool.tile([P, S * DP], dt.float16, name="xn")
                    nc.sync.dma_start(out=xn, in_=xn_view[sidx])
                    xns[sidx] = xn
                    xt = xt_pool.tile([P, 2, W], dt.float16, name="xt")
                    nc.sync.dma_start(out=xt, in_=xt_view4[sidx])
                    xts[sidx] = xt
                return xns[sidx], cc * CH, xts[sidx], cc * CH * P

            def stage_H(c):
                xn, a0, xt, w0 = chunk_src(c)
                h_ps = psum_h.tile([A, CH * P], dt.float32, name="h_ps")
                for h in range(2):
                    nc.tensor.matmul(
                        h_ps,
                        lhsT=w1_sb[:, h, :],
                        rhs=xt[:, h, w0 : w0 + CH * P],
                        start=(h == 0),
                        stop=(h == 1),
                    )
                st[c] = {"xn": xn, "a0": a0, "h_ps": h_ps}

            def stage_Th(c):
                d = st[c]
                th = th_pool.tile([A, CH * P], dt.float16, name="th")
                nc.scalar.activation(
                    th, d.pop("h_ps"), mybir.ActivationFunctionType.Tanh, bias=b1_sb
                )
                d["th"] = th

            def stage_S(c):
                d = st[c]
                st_ps = psum_st.tile([P, CH], dt.float32, name="st_ps")
                th = d.pop("th")
                for j in range(CH):
                    nc.tensor.matmul(
                        st_ps[:, j : j + 1],
                        lhsT=th[:, j * P : (j + 1) * P],
                        rhs=w2_sb,
                        start=True,
                        stop=True,
                        skip_group_check=True,
                    )
                d["st_ps"] = st_ps

            def stage_E(c):
                d = st[c]
                et = et_pool.tile([P, CH], dt.float32, name="et")
                nc.scalar.activation(
                    et, d.pop("st_ps"), mybir.ActivationFunctionType.Exp
                )
                d["et"] = et

            def stage_L(c):
                d = st[c]
                et = d.pop("et")
                sel = sel_pool.tile([P, CH, B], dt.float16, name="sel")
                for j in range(CH):
                    t = c * CH + j
                    nc.vector.tensor_scalar(
                        out=sel[:, j, :],
                        in0=iota_sb,
                        scalar1=seg_sb[:, t : t + 1],
                        scalar2=et[:, j : j + 1],
                        op0=mybir.AluOpType.is_equal,
                        op1=mybir.AluOpType.mult,
                    )
                d["sel"] = sel

            def stage_U(c):
                d = st.pop(c)
                sel = d["sel"]
                xn = d["xn"]
                a0 = d["a0"]
                for j in range(CH):
                    t = c * CH + j
                    nc.tensor.matmul(
                        u_ps,
                        lhsT=sel[:, j, :],
                        rhs=xn[:, (a0 + j) * DP : (a0 + j + 1) * DP],
                        start=(t == 0),
                        stop=(t == T - 1),
                        skip_group_check=True,
                    )

            PRE = 6  # super-DMA prefetch lead (chunks)
            for c in range(TOTCH + 5):
                if c + PRE < TOTCH:
                    chunk_src(c + PRE)
                # DVE first: its deps (E(c-4)) were satisfied an iteration ago
                if 4 <= c < TOTCH + 4:
                    stage_L(c - 4)
                if c < TOTCH:
                    stage_H(c)
                if 1 <= c < TOTCH + 1:
                    stage_Th(c - 1)
                if 2 <= c < TOTCH + 2:
                    stage_S(c - 2)
                if 3 <= c < TOTCH + 3:
                    stage_E(c - 3)
                if 5 <= c:
                    stage_U(c - 5)

            u_sb = out_pool.tile([B, DP], dt.float32)
            nc.vector.tensor_copy(u_sb, u_ps)
            nc.sync.dma_start(out=uout_d.ap(), in_=u_sb)

    nc.compile()
    return nc


def _get_nc():
    if "v4" not in _NC_CACHE:
        _NC_CACHE["v4"] = build_nc()
    return _NC_CACHE["v4"]


def host_inputs(x, segment_ids, W1, b1, w2):
    """Build per-core input maps (shared with test harnesses)."""
    xpad = np.empty((N_TOTAL, DP), dtype=F16)
    xpad[:, :D] = x.astype(F16)
    xpad[:, D] = 1.0
    w1_in = np.ascontiguousarray(W1.astype(F16))
    w2_in = np.ascontiguousarray(w2.astype(F16).reshape(A, 1))
    b1_in = np.ascontiguousarray(b1.astype(F16).reshape(A, 1))
    iota_in = np.broadcast_to(np.arange(B, dtype=F16), (P, B))

    in_maps = []
    for c in range(N_CORES):
        sl = slice(c * R, (c + 1) * R)
        seg_c = np.ascontiguousarray(
            segment_ids[sl]
            .reshape(SUPERS, P, S)
            .transpose(1, 0, 2)
            .reshape(P, T)
            .astype(np.float32)
        )
        # xt[h*128+d, s*W + a*128 + q] = x[(s*128+q)*64 + a, h*128+d]
        xc = xpad[sl, :D].reshape(SUPERS, P, S, 2, P)  # [s, q, a, h, d]
        xt_c = np.ascontiguousarray(
            xc.transpose(3, 4, 0, 2, 1).reshape(2 * P, R)
        )
        in_maps.append(
            {
                "xn": xpad[sl],
                "xt": xt_c,
                "seg": seg_c,
                "w1": w1_in,
                "w2": w2_in,
                "b1": b1_in,
                "iota": iota_in,
            }
        )
    return in_maps


def kernel(x, segment_ids, num_bags, W1, b1, w2, b2, Wh, bh):
    x = np.asarray(x)
    segment_ids = np.asarray(segment_ids)
    W1 = np.asarray(W1)
    b1 = np.asarray(b1)
    w2 = np.asarray(w2)
    Wh = np.asarray(Wh)
    bh = np.asarray(bh)
    num_bags = int(num_bags)
    assert x.shape == (N_TOTAL, D) and num_bags == B

    nc = _get_nc()
    in_maps = host_inputs(x, segment_ids, W1, b1, w2)
    res = run_bass_kernel_spmd(nc, in_maps, core_ids=list(range(N_CORES)))

    U = np.zeros((B, D), np.float64)
    den = np.zeros((B,), np.float64)
    for c in range(N_CORES):
        u = res.results[c]["uout"].astype(np.float64)
        U += u[:, :D]
        den += u[:, D]
    pooled = np.where(den[:, None] > 0, U / np.where(den == 0, 1, den)[:, None], 0.0)
    out = pooled @ Wh.astype(np.float64) + bh.astype(np.float64)
    return out.astype(np.float32)


# revision 4
# speedup vs baseline: 1.1301x; 1.1301x over previous
"""AttentionMIL pooling kernel: dual-layout x + packed constants.

Key facts driving the design (measured on this axon-tunneled setup):
- Per-exec dispatch overhead dominates; each extra ExternalInput tensor
  costs ~40-60us/exec, so all small constants (W1, w2, b1, iota, seg)
  travel in ONE packed fp16 tensor (seg is fp32 bit-packed, bitcast on
  device). 3 inputs total: xn, xt, cst.
- DMA is cheap relative to engine time: the host ships x twice
  (row-major fp16 with a ones column for pooling, feature-major fp16
  for the attention MLP), eliminating all PE transposes and the big
  DVE PSUM->SBUF copies of the earlier kernel.
- Super DMAs are prefetched 12 chunks ahead on two HWDGE queues
  (xn on sync/SP, xt on scalar/ACT) to avoid super-boundary stalls.
- All-fp16 data: fp8 variants measured 1-4.5e-2 rel err (the pooled
  output is a random-walk sum, so per-element quantization noise does
  not average down) for no speed win above measurement noise.

Stage dataflow per chunk (4 subtiles of 128 rows = 512 rows):
  H:  h_ps[a, n] = sum_d W1[d,a] xt[d, n]   (2 K=128 halves into one bank)
  Th: ACT tanh(h_ps + b1) -> th [A, 512] fp16
  S': 4x PE matmuls lhsT=th[:, j*128:(j+1)*128], rhs=w2 [A,1]
      -> st_ps[q, j] = score of (subtile j, row q)  [128, 4]
  E:  ACT exp(st_ps) -> et [128, 4] fp32
  L:  DVE tensor_scalar: sel[q, j, b] = (iota[q,b]==seg[q,t]) * et[q,j]
  U:  4x PE matmuls: u_ps += sel[:,j,:].T @ xn_subtile [128, 257]
      (ones column -> col 256 accumulates the softmax denominator)

Software pipelining: iteration c emits L(c-4) | H(c) Th(c-1) S'(c-2)
E(c-3) U(c-5) so every cross-engine edge has >= 1 chunk of slack.

Host: sum (U, den) partials over cores, pooled = U/den, out = pooled@Wh+bh.
"""

import numpy as np

import concourse.mybir as mybir
import concourse.tile as tile
from concourse import bacc
from concourse.bass_utils import run_bass_kernel_spmd

F16 = np.float16

N_CORES = 8
N_TOTAL = 524288
D = 256
DP = D + 1  # x padded with ones column
A = 128
B = 64  # num bags
P = 128  # SBUF partitions
R = N_TOTAL // N_CORES  # rows per core
T = R // P  # 512 subtiles of 128 rows per core
S = 32  # subtiles per super tile (xn DMA 2.1 MiB, xt DMA 2.0 MiB)
SUPERS = T // S
CH = 4  # subtiles per chunk (512 rows = PSUM bank width)
TOTCH = T // CH  # 128 chunks per core

_NC_CACHE = {}


def build_nc(R=R, S=S, xn_bufs=4, xt_bufs=4, n_cores=N_CORES, debug=False):
    T = R // P
    SUPERS = T // S
    TOTCH = T // CH
    CPS = S // CH  # chunks per super
    W = S * P  # xt columns per super
    dt = mybir.dt
    nc = bacc.Bacc("TRN2", target_bir_lowering=False, debug=debug, num_devices=n_cores)

    CW = 2 * A + 1 + 1 + B + 2 * T  # packed fp16 cols: w1 | w2 | b1 | iota | seg(fp32 bitpacked)
    xn_d = nc.dram_tensor("xn", [R, DP], dt.float16, kind="ExternalInput")
    xt_d = nc.dram_tensor("xt", [2 * P, R], dt.float16, kind="ExternalInput")
    cst_d = nc.dram_tensor("cst", [P, CW], dt.float16, kind="ExternalInput")
    uout_d = nc.dram_tensor("uout", [B, DP], dt.float32, kind="ExternalOutput")

    xn_view = xn_d.ap().rearrange("(s p a) d -> s p (a d)", p=P, a=S)
    xt_view4 = xt_d.ap().rearrange("(h p) (s w) -> s p h w", p=P, s=SUPERS)
    S4 = S // 4  # first super split into 4 quarter DMAs: compute starts sooner
    W4 = W // 4
    cst_view = cst_d.ap()

    with tile.TileContext(nc) as tc:
        with (
            tc.tile_pool(name="persist", bufs=1) as persist,
            tc.tile_pool(name="xn_pool", bufs=xn_bufs) as xn_pool,
            tc.tile_pool(name="xt_pool", bufs=xt_bufs) as xt_pool,
            tc.tile_pool(name="th_pool", bufs=3) as th_pool,
            tc.tile_pool(name="et_pool", bufs=3) as et_pool,
            tc.tile_pool(name="sel_pool", bufs=3) as sel_pool,
            tc.tile_pool(name="out_pool", bufs=1) as out_pool,
            tc.tile_pool(name="psum_u", bufs=1, space="PSUM") as psum_u,
            tc.tile_pool(name="psum_h", bufs=2, space="PSUM") as psum_h,
            tc.tile_pool(name="psum_st", bufs=2, space="PSUM") as psum_st,
        ):
            cst_sb = persist.tile([P, 2 * A + 2 + B + 2 * T], dt.float16)
            nc.sync.dma_start(out=cst_sb, in_=cst_view)
            w1_sb = cst_sb[:, 0 : 2 * A].rearrange("p (h a) -> p h a", h=2)
            w2_sb = cst_sb[:, 2 * A : 2 * A + 1]
            b1_sb = cst_sb[:, 2 * A + 1 : 2 * A + 2]
            iota_sb = cst_sb[:, 2 * A + 2 : 2 * A + 2 + B]
            seg_sb = cst_sb[:, 2 * A + 2 + B :].bitcast(dt.float32)

            u_ps = psum_u.tile([B, DP], dt.float32)

            xns = {}  # super idx -> xn tile
            xts = {}  # super idx -> xt tile
            xn0 = []  # super 0 quarter tiles
            xt0 = []
            for k in range(4):
                q = persist.tile([P, S4 * DP], dt.float16, name=f"xn0_{k}")
                nc.sync.dma_start(
                    out=q, in_=xn_view[0][:, k * S4 * DP : (k + 1) * S4 * DP]
                )
                xn0.append(q)
                qt = persist.tile([P, 2, W4], dt.float16, name=f"xt0_{k}")
                nc.scalar.dma_start(
                    out=qt, in_=xt_view4[0][:, :, k * W4 : (k + 1) * W4]
                )
                xt0.append(qt)
            st = {}  # chunk idx -> per-stage state

            def chunk_src(c):
                """(xn tile, subtile off, xt tile, col off) for chunk c."""
                sidx = c // CPS
                cc = c % CPS
                if sidx == 0:
                    per_q = CPS // 4  # chunks per quarter of super 0
                    qq = cc // per_q
                    a0 = (cc % per_q) * CH
                    return xn0[qq], a0, xt0[qq], a0 * P
                if sidx not in xns:
                    xn = xn_pool.tile([P, S * DP], dt.float16, name="xn")
                    nc.sync.dma_start(out=xn, in_=xn_view[sidx])
                    xns[sidx] = xn
                    xt = xt_pool.tile([P, 2, W], dt.float16, name="xt")
                    nc.scalar.dma_start(out=xt, in_=xt_view4[sidx])
                    xts[sidx] = xt
                return xns[sidx], cc * CH, xts[sidx], cc * CH * P

            def stage_H(c):
                xn, a0, xt, w0 = chunk_src(c)
                h_ps = psum_h.tile([A, CH * P], dt.float32, name="h_ps")
                for h in range(2):
                    nc.tensor.matmul(
                        h_ps,
                        lhsT=w1_sb[:, h, :],
                        rhs=xt[:, h, w0 : w0 + CH * P],
                        start=(h == 0),
                        stop=(h == 1),
                    )
                st[c] = {"xn": xn, "a0": a0, "h_ps": h_ps}

            def stage_Th(c):
                d = st[c]
                th = th_pool.tile([A, CH * P], dt.float16, name="th")
                nc.scalar.activation(
                    th, d.pop("h_ps"), mybir.ActivationFunctionType.Tanh, bias=b1_sb
                )
                d["th"] = th

            def stage_S(c):
                d = st[c]
                st_ps = psum_st.tile([P, CH], dt.float32, name="st_ps")
                th = d.pop("th")
                for j in range(CH):
                    nc.tensor.matmul(
                        st_ps[:, j : j + 1],
                        lhsT=th[:, j * P : (j + 1) * P],
                        rhs=w2_sb,
                        start=True,
                        stop=True,
                        skip_group_check=True,
                    )
                d["st_ps"] = st_ps

            def stage_E(c):
                d = st[c]
                et = et_pool.tile([P, CH], dt.float32, name="et")
                nc.scalar.activation(
                    et, d.pop("st_ps"), mybir.ActivationFunctionType.Exp
                )
                d["et"] = et

            def stage_L(c):
                d = st[c]
                et = d.pop("et")
                sel = sel_pool.tile([P, CH, B], dt.float16, name="sel")
                for j in range(CH):
                    t = c * CH + j
                    nc.vector.tensor_scalar(
                        out=sel[:, j, :],
                        in0=iota_sb,
                        scalar1=seg_sb[:, t : t + 1],
                        scalar2=et[:, j : j + 1],
                        op0=mybir.AluOpType.is_equal,
                        op1=mybir.AluOpType.mult,
                    )
                d["sel"] = sel

            def stage_U(c):
                d = st.pop(c)
                sel = d["sel"]
                xn = d["xn"]
                a0 = d["a0"]
                for j in range(CH):
                    t = c * CH + j
                    nc.tensor.matmul(
                        u_ps,
                        lhsT=sel[:, j, :],
                        rhs=xn[:, (a0 + j) * DP : (a0 + j + 1) * DP],
                        start=(t == 0),
                        stop=(t == T - 1),
                        skip_group_check=True,
                    )

            PRE = 12  # super-DMA prefetch lead (chunks)
            for c in range(TOTCH + 5):
                if c + PRE < TOTCH:
                    chunk_src(c + PRE)
                # DVE first: its deps (E(c-4)) were satisfied an iteration ago
                if 4 <= c < TOTCH + 4:
                    stage_L(c - 4)
                if c < TOTCH:
                    stage_H(c)
                if 1 <= c < TOTCH + 1:
                    stage_Th(c - 1)
                if 2 <= c < TOTCH + 2:
                    stage_S(c - 2)
                if 3 <= c < TOTCH + 3:
                    stage_E(c - 3)
                if 5 <= c:
                    stage_U(c - 5)

            u_sb = out_pool.tile([B, DP], dt.float32)
            nc.vector.tensor_copy(u_sb, u_ps)
            nc.sync.dma_start(out=uout_d.ap(), in_=u_sb)

    nc.compile()
    return nc


def _get_nc():
    if "v4" not in _NC_CACHE:
        _NC_CACHE["v4"] = build_nc()
    return _NC_CACHE["v4"]


def host_inputs(x, segment_ids, W1, b1, w2):
    """Build per-core input maps (shared with test harnesses)."""
    xpad = np.empty((N_TOTAL, DP), dtype=F16)
    xpad[:, :D] = x.astype(F16)
    xpad[:, D] = 1.0
    w1_pk = W1.astype(F16).reshape(2, P, A).transpose(1, 0, 2).reshape(P, 2 * A)
    w2_pk = w2.astype(F16).reshape(P, 1)
    b1_pk = b1.astype(F16).reshape(P, 1)
    iota_pk = np.broadcast_to(np.arange(B, dtype=F16), (P, B))

    in_maps = []
    for c in range(N_CORES):
        sl = slice(c * R, (c + 1) * R)
        seg_c = np.ascontiguousarray(
            segment_ids[sl]
            .reshape(SUPERS, P, S)
            .transpose(1, 0, 2)
            .reshape(P, T)
            .astype(np.float32)
        ).view(F16)
        cst_c = np.ascontiguousarray(
            np.concatenate([w1_pk, w2_pk, b1_pk, iota_pk, seg_c], axis=1)
        )
        # xt[h*128+d, s*W + a*128 + q] = x[(s*128+q)*64 + a, h*128+d]
        xc = xpad[sl, :D].reshape(SUPERS, P, S, 2, P)  # [s, q, a, h, d]
        xt_c = np.ascontiguousarray(
            xc.transpose(3, 4, 0, 2, 1).reshape(2 * P, R)
        )
        in_maps.append({"xn": xpad[sl], "xt": xt_c, "cst": cst_c})
    return in_maps


def kernel(x, segment_ids, num_bags, W1, b1, w2, b2, Wh, bh):
    x = np.asarray(x)
    segment_ids = np.asarray(segment_ids)
    W1 = np.asarray(W1)
    b1 = np.asarray(b1)
    w2 = np.asarray(w2)
    Wh = np.asarray(Wh)
    bh = np.asarray(bh)
    num_bags = int(num_bags)
    assert x.shape == (N_TOTAL, D) and num_bags == B

    nc = _get_nc()
    in_maps = host_inputs(x, segment_ids, W1, b1, w2)
    res = run_bass_kernel_spmd(nc, in_maps, core_ids=list(range(N_CORES)))

    U = np.zeros((B, D), np.float64)
    den = np.zeros((B,), np.float64)
    for c in range(N_CORES):
        u = res.results[c]["uout"].astype(np.float64)
        U += u[:, :D]
        den += u[:, D]
    pooled = np.where(den[:, None] > 0, U / np.where(den == 0, 1, den)[:, None], 0.0)
    out = pooled @ Wh.astype(np.float64) + bh.astype(np.float64)
    return out.astype(np.float32)
